# revision 14
# baseline (speedup 1.0000x reference)
"""GCN encoder (2-layer) Bass kernel for Trainium2, 8 NeuronCores.

Strategy (graph/data parallel; dest nodes sharded, contiguous ranges):
  - Nodes padded to NPAD=50176; core c owns dest blocks [c*49, (c+1)*49),
    49 blocks of 128 dests each.
  - Edges (incl. self-loops) bucketed by (dest block, src half); each bucket
    padded to TH tiles of 128 edge slots. All normalization (dinv[src] *
    w * dinv[dst]) is folded into host-precomputed one-hot tiles
    oh[slot, dst_col] (bf16), shared by both layers.
  - Layer 1 needs no runtime gather: x is a kernel input, so the per-edge
    source rows xg[slot] = x[src] are gathered ON HOST and streamed as
    dense tiles. Per dest block: aggT[f, d] = sum_t xg_tile^T oh_tile
    (PE, 2 fin chunks), then out1T[h, d] = W1c^T aggT (PE), bias+relu on
    ACT (b1 is per-partition in this transposed layout), then
    hs2[d, f2] = reluT^T W2c (PE). No transposes needed anywhere.
  - hs2 shards exchanged with AllGather; layer 2 fetches per-edge rows with
    dma_gather (128 rows/tile) and aggregates with the same one-hot tiles:
    out2T[f2, d] = sum msg^T oh. Output is written f2-major and transposed
    on host.

kernel(**inputs) takes FULL inputs, returns the FULL [50000,128] f32 output.
"""

import sys

sys.path.insert(0, "/opt/trn_rl_repo")

import numpy as np
import ml_dtypes

P = 128
NCORES = 8
BPC = 49                 # dest blocks per core
SHARD = BPC * P          # 6272
NPAD = NCORES * SHARD    # 50176
HALF = NPAD // 2         # 25088
NB = NPAD // P           # 392 dest blocks
N = 50000
FIN = 256
H = 256                  # layer-1 output width
F2 = 128                 # layer-2 output width

_BF16 = ml_dtypes.bfloat16


def _preprocess(edge_index, edge_weight):
    """Edge bucketing + all graph-structure-derived device arrays."""
    row = np.asarray(edge_index[0], dtype=np.int64)
    col = np.asarray(edge_index[1], dtype=np.int64)
    w = np.asarray(edge_weight, dtype=np.float32)

    loop = np.arange(N, dtype=np.int64)
    rows = np.concatenate([row, loop])
    cols = np.concatenate([col, loop])
    ws = np.concatenate([w, np.ones(N, np.float32)])
    EE = rows.shape[0]

    deg = np.bincount(cols, weights=ws.astype(np.float64), minlength=NPAD)
    deg = deg.astype(np.float32)
    dinv = np.where(deg > 0, 1.0 / np.sqrt(np.maximum(deg, 1e-30)), 0.0)
    dinv = dinv.astype(np.float32)
    wfull = (dinv[rows] * ws * dinv[cols]).astype(np.float32)

    blk = cols // P
    half = (rows >= HALF).astype(np.int64)
    key = blk * 2 + half                    # bucket id in [0, NB*2)

    # Sort by (bucket, src, col); fold duplicate (bucket,src,col) weights;
    # dedup (bucket, src) into slots so each distinct source is gathered once
    # per bucket (its oh row then has one entry per incident dest col).
    skey = (key * NPAD + rows) * P + (cols % P)
    order = np.argsort(skey, kind="stable")
    sk = skey[order]
    w_s = wfull[order]
    grp_first = np.ones(EE, bool)
    grp_first[1:] = sk[1:] != sk[:-1]
    gidx = np.cumsum(grp_first) - 1
    w_g = np.bincount(gidx, weights=w_s.astype(np.float64)).astype(np.float32)
    sk_g = sk[grp_first]                    # unique (bucket,src,col), sorted
    col_g = sk_g % P
    bs_g = sk_g // P                        # bucket*NPAD + src
    key_g = bs_g // NPAD
    src_g = bs_g % NPAD

    slot_first = np.ones(bs_g.shape[0], bool)
    slot_first[1:] = bs_g[1:] != bs_g[:-1]
    slot_id = np.cumsum(slot_first) - 1     # global slot per (bucket,src)
    bs_u = bs_g[slot_first]
    key_u = bs_u // NPAD
    src_u = bs_u % NPAD
    ucnt = np.bincount(key_u, minlength=NB * 2)   # unique srcs per bucket
    TH = int(-(-ucnt.max() // P))
    CAP = TH * P
    ucs = np.zeros(NB * 2 + 1, np.int64)
    np.cumsum(ucnt, out=ucs[1:])
    upos = np.arange(bs_u.shape[0]) - ucs[key_u]  # slot pos within bucket

    # per-slot src array, bucket-major (pad = -1)
    src_slot = np.full(NB * 2 * CAP, -1, np.int64)
    src_slot[key_u * CAP + upos] = src_u

    # one-hot tiles [NB*2*TH, 128 slot, 128 dst] bf16, weights folded
    oh = np.zeros((NB * 2 * TH, P, P), _BF16)
    up = upos[slot_id]                            # within-bucket slot of entry
    oh[key_g * TH + up // P, up % P, col_g] = w_g.astype(_BF16)

    # gather indices, int16, relative to half, positive pads (row 0).
    IW = CAP // 16
    src_rel = np.where(src_slot >= 0,
                       np.where(src_slot >= HALF, src_slot - HALF, src_slot),
                       0).astype(np.int16)
    idx_w = src_rel.reshape(NB, 2, IW, 16).transpose(0, 1, 3, 2)
    idx_w = np.ascontiguousarray(np.tile(idx_w, (1, 1, 8, 1)))  # [NB,2,128,IW]

    return dict(TH=TH, CAP=CAP, oh=oh, src_slot=src_slot, idx_w=idx_w,
                dinv=dinv)


def _host_golden(x, W1, b1, W2, b2, pp):
    """Numpy re-implementation of the device algorithm with the same bf16
    quantization points, for off-device validation."""
    bf = lambda a: a.astype(_BF16).astype(np.float32)
    TH, CAP = pp["TH"], pp["CAP"]
    oh = pp["oh"].astype(np.float32)         # [NB*2*TH, 128, 128]
    src = pp["src_slot"].reshape(NB, 2 * CAP)

    xp = np.zeros((NPAD, FIN), np.float32)
    xp[:N] = x
    xbf = bf(xp)
    W1b, W2b = bf(W1), bf(W2)

    hs2 = np.zeros((NPAD, F2), np.float32)
    for nb in range(NB):
        aggT = np.zeros((FIN, P), np.float32)
        for t in range(2 * TH):
            sl = src[nb, t * P:(t + 1) * P]
            xg = xbf[np.clip(sl, 0, None)]          # [128, 256]
            o = oh[nb * 2 * TH + t]                 # [128 slot, 128 dst]
            aggT += xg.T @ o
        aggT = bf(aggT)
        out1T = W1b.T @ aggT + b1[:, None]          # [256 h, 128 d]
        reluT = bf(np.maximum(out1T, 0.0))
        hs2[nb * P:(nb + 1) * P] = bf(reluT.T @ W2b)

    hs2b = bf(hs2)
    out = np.zeros((NPAD, F2), np.float32)
    for nb in range(NB):
        out2T = np.zeros((F2, P), np.float32)
        for t in range(2 * TH):
            sl = src[nb, t * P:(t + 1) * P]
            msg = hs2b[np.clip(sl, 0, None)]        # [128, 128]
            o = oh[nb * 2 * TH + t]
            out2T += msg.T @ o
        out[nb * P:(nb + 1) * P] = out2T.T + b2[None, :]
    return out[:N].astype(np.float32)


# ---------------------------------------------------------------------------
# Bass device kernel
# ---------------------------------------------------------------------------

_NC_CACHE = {}


def _build_nc(TH):
    import concourse.bass as bass  # noqa: F401
    import concourse.mybir as mybir
    import concourse.tile as tile
    from concourse import bacc
    from concourse.library_config import mlp

    DT = mybir.dt.bfloat16
    F32 = mybir.dt.float32
    I16 = mybir.dt.int16
    I32 = mybir.dt.int32
    AF = mybir.ActivationFunctionType
    AL = mybir.AluOpType

    CAP = TH * P
    IW = CAP // 16
    NT = 2 * TH              # tiles per dest block
    GSZ = 4                  # dest blocks per merged gather

    nc = bacc.Bacc("TRN2", target_bir_lowering=False, debug=True,
                   num_devices=NCORES)
    xg_d = nc.dram_tensor("xg", [P, BPC * NT * FIN], DT, kind="ExternalInput")
    oh_d = nc.dram_tensor("oh", [P, BPC * NT * P], DT, kind="ExternalInput")
    idx_d = nc.dram_tensor("idxP", [P, BPC * 2 * IW], I16, kind="ExternalInput")
    w1_d = nc.dram_tensor("w1c", [P, 2 * 2 * P], DT, kind="ExternalInput")
    w2_d = nc.dram_tensor("w2c", [P, 2 * F2], DT, kind="ExternalInput")
    b1_d = nc.dram_tensor("b1c", [P, 2], F32, kind="ExternalInput")
    b2_d = nc.dram_tensor("b2c", [P, 1], F32, kind="ExternalInput")
    out_d = nc.dram_tensor("out2T", [P, BPC * P], F32, kind="ExternalOutput")

    with tile.TileContext(nc) as tc:
        with (
            tc.tile_pool(name="dram", bufs=1, space="DRAM") as dpool,
            tc.tile_pool(name="const", bufs=1) as cpool,
            tc.tile_pool(name="stream", bufs=4) as xpool,
            tc.tile_pool(name="ohs", bufs=2) as opool,
            tc.tile_pool(name="sb", bufs=3) as spool,
            tc.tile_pool(name="pagg", bufs=4, space="PSUM") as pagg,
            tc.tile_pool(name="po1", bufs=2, space="PSUM") as po1,
            tc.tile_pool(name="psm", bufs=2, space="PSUM") as psm,
        ):
            hs2_shard = dpool.tile([SHARD, F2], DT)
            hs2_full = dpool.tile([NPAD, F2], DT, addr_space="Shared")

            nc.gpsimd.load_library(mlp)

            # ---- constants ----
            w1_sb = cpool.tile([P, 2, 2, P], DT)     # [fin_c, h_c]
            nc.sync.dma_start(out=w1_sb[:], in_=w1_d[:])
            w2_sb = cpool.tile([P, 2, F2], DT)       # [h_c]
            nc.sync.dma_start(out=w2_sb[:], in_=w2_d[:])
            b1_sb = cpool.tile([P, 2], F32)
            nc.sync.dma_start(out=b1_sb[:], in_=b1_d[:])
            b2_sb = cpool.tile([P, 1], F32)
            nc.sync.dma_start(out=b2_sb[:], in_=b2_d[:])
            idx_sb = cpool.tile([P, BPC * 2 * IW], I16)
            nc.sync.dma_start(out=idx_sb[:], in_=idx_d[:])

            # ---- layer 1 + layer-2 table, per dest block ----
            for b in range(BPC):
                xg = xpool.tile([P, NT, FIN], DT, tag="stream")
                nc.sync.dma_start(
                    out=xg[:], in_=xg_d[:, b * NT * FIN:(b + 1) * NT * FIN])
                oh = opool.tile([P, NT, P], DT, tag="oh")
                nc.sync.dma_start(
                    out=oh[:], in_=oh_d[:, b * NT * P:(b + 1) * NT * P])

                aggT0 = pagg.tile([P, P], F32, tag="aggT")
                aggT1 = pagg.tile([P, P], F32, tag="aggT")
                aggTp = [aggT0, aggT1]
                for t in range(NT):
                    for c in range(2):
                        nc.tensor.matmul(
                            aggTp[c][:],
                            lhsT=xg[:, t, c * P:(c + 1) * P],
                            rhs=oh[:, t, :],
                            start=(t == 0), stop=(t == NT - 1))
                aggT_sb = spool.tile([P, 2, P], DT, tag="aggT_sb")
                for c in range(2):
                    nc.vector.tensor_copy(aggT_sb[:, c, :], aggTp[c][:])

                reluT_sb = spool.tile([P, 2, P], DT, tag="reluT")
                for hc in range(2):
                    o1 = po1.tile([P, P], F32, tag="o1")
                    for c in range(2):
                        nc.tensor.matmul(
                            o1[:], lhsT=w1_sb[:, c, hc, :],
                            rhs=aggT_sb[:, c, :],
                            start=(c == 0), stop=(c == 1))
                    nc.scalar.activation(reluT_sb[:, hc, :], o1[:], AF.Relu,
                                         bias=b1_sb[:, hc:hc + 1])

                ph = psm.tile([P, F2], F32, tag="sm")
                for hc in range(2):
                    nc.tensor.matmul(ph[:], lhsT=reluT_sb[:, hc, :],
                                     rhs=w2_sb[:, hc, :],
                                     start=(hc == 0), stop=(hc == 1))
                hsb = spool.tile([P, F2], DT, tag="hsb")
                nc.vector.tensor_copy(hsb[:], ph[:])
                nc.sync.dma_start(out=hs2_shard[b * P:(b + 1) * P, :],
                                  in_=hsb[:])

            # ---- exchange hs2 shards ----
            nc.gpsimd.collective_compute(
                "AllGather", AL.bypass,
                replica_groups=[list(range(NCORES))],
                ins=[hs2_shard[:]],
                outs=[hs2_full[:]],
            )

            # ---- layer 2: merged gathers per (group of GSZ blocks, half) ----
            for g0 in range(0, BPC, GSZ):
                gsz = min(GSZ, BPC - g0)
                msgs = []
                for hh in range(2):
                    m = xpool.tile([P, gsz * TH, F2], DT, tag="stream")
                    src = hs2_full[0:HALF, :] if hh == 0 else hs2_full[HALF:NPAD, :]
                    k = hh * BPC + g0
                    nc.gpsimd.dma_gather(
                        m[:], src,
                        idx_sb[:, k * IW:(k + gsz) * IW],
                        gsz * CAP, gsz * CAP, F2, single_packet=False)
                    msgs.append(m)
                for bb in range(gsz):
                    b = g0 + bb
                    oh2 = opool.tile([P, NT, P], DT, tag="oh")
                    nc.sync.dma_start(
                        out=oh2[:], in_=oh_d[:, b * NT * P:(b + 1) * NT * P])
                    p2 = psm.tile([P, P], F32, tag="sm")
                    for t in range(NT):
                        hh, tt = (0, t) if t < TH else (1, t - TH)
                        nc.tensor.matmul(
                            p2[:], lhsT=msgs[hh][:, bb * TH + tt, :],
                            rhs=oh2[:, t, :],
                            start=(t == 0), stop=(t == NT - 1))
                    o2 = spool.tile([P, P], F32, tag="o2")
                    nc.vector.tensor_scalar(o2[:], p2[:], b2_sb[:, 0:1], None,
                                            AL.add)
                    nc.sync.dma_start(out=out_d[:, b * P:(b + 1) * P],
                                      in_=o2[:])

    nc.compile()
    return nc


def _make_inputs(x, W1, b1, W2, b2, pp):
    TH = pp["TH"]
    CAP = TH * P
    NT = 2 * TH
    IW = CAP // 16

    xp = np.zeros((NPAD, FIN), np.float32)
    xp[:N] = x
    xbf = xp.astype(_BF16)

    # weights in device layouts
    w1c = np.ascontiguousarray(
        W1.astype(_BF16).reshape(2, P, 2, P).transpose(1, 0, 2, 3)
    ).reshape(P, 2 * 2 * P)           # [p, fin_c, h_c, h_lo]
    w2c = np.ascontiguousarray(
        W2.astype(_BF16).reshape(2, P, F2).transpose(1, 0, 2)
    ).reshape(P, 2 * F2)              # [p, h_c, f2]
    b1c = np.ascontiguousarray(
        b1.astype(np.float32).reshape(2, P).T)       # [p, h_c]
    b2c = np.ascontiguousarray(
        b2.astype(np.float32).reshape(P, 1))

    oh = pp["oh"]                     # [NB*2*TH, 128, 128] bf16
    src_slot = pp["src_slot"]         # [NB*2*CAP]
    idx_w = pp["idx_w"]               # [NB, 2, 128, IW]

    in_maps = []
    for c in range(NCORES):
        b0 = c * BPC
        sl = src_slot[b0 * 2 * CAP:(b0 + BPC) * 2 * CAP]
        xg = xbf[np.clip(sl, 0, None)]               # [BPC*NT*128, 256]
        xg = np.ascontiguousarray(
            xg.reshape(BPC * NT, P, FIN).transpose(1, 0, 2)
        ).reshape(P, BPC * NT * FIN)
        ohc = np.ascontiguousarray(
            oh[b0 * NT:(b0 + BPC) * NT].transpose(1, 0, 2)
        ).reshape(P, BPC * NT * P)
        idxc = np.ascontiguousarray(
            idx_w[b0:b0 + BPC].transpose(2, 1, 0, 3)
        ).reshape(P, 2 * BPC * IW)
        in_maps.append({
            "xg": xg, "oh": ohc, "idxP": idxc,
            "w1c": w1c, "w2c": w2c, "b1c": b1c, "b2c": b2c,
        })
    return in_maps


def kernel(x, edge_index, edge_weight, W1, b1, W2, b2, _trace=False):
    from concourse.bass_utils import run_bass_kernel_spmd

    x = np.asarray(x, dtype=np.float32)
    W1 = np.asarray(W1, dtype=np.float32)
    b1 = np.asarray(b1, dtype=np.float32)
    W2 = np.asarray(W2, dtype=np.float32)
    b2 = np.asarray(b2, dtype=np.float32)

    pp = _preprocess(np.asarray(edge_index), np.asarray(edge_weight))
    key = (pp["TH"],)
    if key not in _NC_CACHE:
        _NC_CACHE[key] = _build_nc(*key)
    nc = _NC_CACHE[key]

    in_maps = _make_inputs(x, W1, b1, W2, b2, pp)
    res = run_bass_kernel_spmd(nc, in_maps, list(range(NCORES)), trace=_trace)
    out = np.concatenate(
        [np.asarray(res.results[c]["out2T"]).T for c in range(NCORES)], axis=0)
    if _trace:
        kernel._last_result = res
    return np.ascontiguousarray(out[:N].astype(np.float32))


# revision 15
# speedup vs baseline: 1.2206x; 1.2206x over previous
"""GCN encoder (2-layer) Bass kernel for Trainium2, 8 NeuronCores.

Strategy (graph/data parallel; dest nodes sharded, contiguous ranges):
  - Nodes padded to NPAD=50176; core c owns dest blocks [c*49, (c+1)*49),
    49 blocks of 128 dests each.
  - Edges (incl. self-loops) bucketed by (dest block, src half); each bucket
    padded to TH tiles of 128 edge slots. All normalization (dinv[src] *
    w * dinv[dst]) is folded into host-precomputed one-hot tiles
    oh[slot, dst_col] (bf16), shared by both layers.
  - Layer 1 needs no runtime gather: x is a kernel input, so the per-edge
    source rows xg[slot] = x[src] are gathered ON HOST and streamed as
    dense tiles. Per dest block: aggT[f, d] = sum_t xg_tile^T oh_tile
    (PE, 2 fin chunks), then out1T[h, d] = W1c^T aggT (PE), bias+relu on
    ACT (b1 is per-partition in this transposed layout), then
    hs2[d, f2] = reluT^T W2c (PE). No transposes needed anywhere.
  - hs2 shards exchanged with AllGather; layer 2 fetches per-edge rows with
    dma_gather (128 rows/tile) and aggregates with the same one-hot tiles:
    out2T[f2, d] = sum msg^T oh. Output is written f2-major and transposed
    on host.

kernel(**inputs) takes FULL inputs, returns the FULL [50000,128] f32 output.
"""

import sys

sys.path.insert(0, "/opt/trn_rl_repo")

import numpy as np
import ml_dtypes

P = 128
NCORES = 8
BPC = 49                 # dest blocks per core
SHARD = BPC * P          # 6272
NPAD = NCORES * SHARD    # 50176
HALF = NPAD // 2         # 25088
NB = NPAD // P           # 392 dest blocks
N = 50000
FIN = 256
H = 256                  # layer-1 output width
F2 = 128                 # layer-2 output width

_BF16 = ml_dtypes.bfloat16


def _preprocess(edge_index, edge_weight):
    """Edge bucketing + all graph-structure-derived device arrays."""
    row = np.asarray(edge_index[0], dtype=np.int64)
    col = np.asarray(edge_index[1], dtype=np.int64)
    w = np.asarray(edge_weight, dtype=np.float32)

    loop = np.arange(N, dtype=np.int64)
    rows = np.concatenate([row, loop])
    cols = np.concatenate([col, loop])
    ws = np.concatenate([w, np.ones(N, np.float32)])
    EE = rows.shape[0]

    deg = np.bincount(cols, weights=ws.astype(np.float64), minlength=NPAD)
    deg = deg.astype(np.float32)
    dinv = np.where(deg > 0, 1.0 / np.sqrt(np.maximum(deg, 1e-30)), 0.0)
    dinv = dinv.astype(np.float32)
    wfull = (dinv[rows] * ws * dinv[cols]).astype(np.float32)

    blk = cols // P
    half = (rows >= HALF).astype(np.int64)
    key = blk * 2 + half                    # bucket id in [0, NB*2)

    # Sort by (bucket, src, col); fold duplicate (bucket,src,col) weights;
    # dedup (bucket, src) into slots so each distinct source is gathered once
    # per bucket (its oh row then has one entry per incident dest col).
    skey = (key * NPAD + rows) * P + (cols % P)
    order = np.argsort(skey, kind="stable")
    sk = skey[order]
    w_s = wfull[order]
    grp_first = np.ones(EE, bool)
    grp_first[1:] = sk[1:] != sk[:-1]
    gidx = np.cumsum(grp_first) - 1
    w_g = np.bincount(gidx, weights=w_s.astype(np.float64)).astype(np.float32)
    sk_g = sk[grp_first]                    # unique (bucket,src,col), sorted
    col_g = sk_g % P
    bs_g = sk_g // P                        # bucket*NPAD + src
    key_g = bs_g // NPAD
    src_g = bs_g % NPAD

    slot_first = np.ones(bs_g.shape[0], bool)
    slot_first[1:] = bs_g[1:] != bs_g[:-1]
    slot_id = np.cumsum(slot_first) - 1     # global slot per (bucket,src)
    bs_u = bs_g[slot_first]
    key_u = bs_u // NPAD
    src_u = bs_u % NPAD
    ucnt = np.bincount(key_u, minlength=NB * 2)   # unique srcs per bucket
    TH = int(-(-ucnt.max() // P))
    CAP = TH * P
    ucs = np.zeros(NB * 2 + 1, np.int64)
    np.cumsum(ucnt, out=ucs[1:])
    upos = np.arange(bs_u.shape[0]) - ucs[key_u]  # slot pos within bucket

    # per-slot src array, bucket-major (pad = -1)
    src_slot = np.full(NB * 2 * CAP, -1, np.int64)
    src_slot[key_u * CAP + upos] = src_u

    # one-hot tiles [NB*2*TH, 128 slot, 128 dst] bf16, weights folded
    oh = np.zeros((NB * 2 * TH, P, P), _BF16)
    up = upos[slot_id]                            # within-bucket slot of entry
    oh[key_g * TH + up // P, up % P, col_g] = w_g.astype(_BF16)

    # gather indices, int16, relative to half. Per-(block,half) gather
    # count is static across cores: max real count over the 8 cores; slots
    # between a core's real count and the static count are positive 0-pads
    # (w=0), beyond it -1 (descriptors trimmed by the ucode; ring space is
    # reserved from the static count). First 2 blocks use full CAP so the
    # msg-tile pool slots get fully written on first touch.
    IW = CAP // 16
    src_rel = np.where(src_slot >= 0,
                       np.where(src_slot >= HALF, src_slot - HALF, src_slot),
                       -1).astype(np.int16)
    src_rel = src_rel.reshape(NB * 2, CAP)
    ucnt2 = ucnt.reshape(NCORES, BPC * 2)
    cnts = ucnt2.max(axis=0).astype(np.int32)       # [BPC*2]
    cnts[:2 * 2] = CAP
    for c in range(NCORES):
        for k2 in range(BPC * 2):
            k = c * BPC * 2 + k2
            row = src_rel[k]
            row[ucnt2[c, k2]:cnts[k2]] = 0
    idx_w = src_rel.reshape(NB, 2, IW, 16).transpose(0, 1, 3, 2)
    idx_w = np.ascontiguousarray(np.tile(idx_w, (1, 1, 8, 1)))  # [NB,2,128,IW]

    return dict(TH=TH, CAP=CAP, oh=oh, src_slot=src_slot, idx_w=idx_w,
                cnts=cnts, dinv=dinv)


def _host_golden(x, W1, b1, W2, b2, pp):
    """Numpy re-implementation of the device algorithm with the same bf16
    quantization points, for off-device validation."""
    bf = lambda a: a.astype(_BF16).astype(np.float32)
    TH, CAP = pp["TH"], pp["CAP"]
    oh = pp["oh"].astype(np.float32)         # [NB*2*TH, 128, 128]
    src = pp["src_slot"].reshape(NB, 2 * CAP)

    xp = np.zeros((NPAD, FIN), np.float32)
    xp[:N] = x
    xbf = bf(xp)
    W1b, W2b = bf(W1), bf(W2)

    hs2 = np.zeros((NPAD, F2), np.float32)
    for nb in range(NB):
        aggT = np.zeros((FIN, P), np.float32)
        for t in range(2 * TH):
            sl = src[nb, t * P:(t + 1) * P]
            xg = xbf[np.clip(sl, 0, None)]          # [128, 256]
            o = oh[nb * 2 * TH + t]                 # [128 slot, 128 dst]
            aggT += xg.T @ o
        aggT = bf(aggT)
        out1T = W1b.T @ aggT + b1[:, None]          # [256 h, 128 d]
        reluT = bf(np.maximum(out1T, 0.0))
        hs2[nb * P:(nb + 1) * P] = bf(reluT.T @ W2b)

    hs2b = bf(hs2)
    out = np.zeros((NPAD, F2), np.float32)
    for nb in range(NB):
        out2T = np.zeros((F2, P), np.float32)
        for t in range(2 * TH):
            sl = src[nb, t * P:(t + 1) * P]
            msg = hs2b[np.clip(sl, 0, None)]        # [128, 128]
            o = oh[nb * 2 * TH + t]
            out2T += msg.T @ o
        out[nb * P:(nb + 1) * P] = out2T.T + b2[None, :]
    return out[:N].astype(np.float32)


# ---------------------------------------------------------------------------
# Bass device kernel
# ---------------------------------------------------------------------------

_NC_CACHE = {}


def _build_nc(TH, cnts):
    import concourse.bass as bass  # noqa: F401
    import concourse.mybir as mybir
    import concourse.tile as tile
    from concourse import bacc
    from concourse.library_config import mlp

    DT = mybir.dt.bfloat16
    F32 = mybir.dt.float32
    I16 = mybir.dt.int16
    I32 = mybir.dt.int32
    AF = mybir.ActivationFunctionType
    AL = mybir.AluOpType

    CAP = TH * P
    IW = CAP // 16
    NT = 2 * TH              # tiles per dest block

    nc = bacc.Bacc("TRN2", target_bir_lowering=False, debug=True,
                   num_devices=NCORES)
    xg_d = nc.dram_tensor("xg", [P, BPC * NT * FIN], DT, kind="ExternalInput")
    oh_d = nc.dram_tensor("oh", [P, BPC * NT * P], DT, kind="ExternalInput")
    idx_d = nc.dram_tensor("idxP", [P, BPC * 2 * IW], I16, kind="ExternalInput")
    w1_d = nc.dram_tensor("w1c", [P, 2 * 2 * P], DT, kind="ExternalInput")
    w2_d = nc.dram_tensor("w2c", [P, 2 * F2], DT, kind="ExternalInput")
    b1_d = nc.dram_tensor("b1c", [P, 2], F32, kind="ExternalInput")
    b2_d = nc.dram_tensor("b2c", [P, 1], F32, kind="ExternalInput")
    out_d = nc.dram_tensor("out2T", [P, BPC * P], F32, kind="ExternalOutput")

    with tile.TileContext(nc) as tc:
        with (
            tc.tile_pool(name="dram", bufs=1, space="DRAM") as dpool,
            tc.tile_pool(name="const", bufs=1) as cpool,
            tc.tile_pool(name="stream", bufs=4) as xpool,
            tc.tile_pool(name="ohs", bufs=2) as opool,
            tc.tile_pool(name="sb", bufs=3) as spool,
            tc.tile_pool(name="pagg", bufs=4, space="PSUM") as pagg,
            tc.tile_pool(name="po1", bufs=2, space="PSUM") as po1,
            tc.tile_pool(name="psm", bufs=2, space="PSUM") as psm,
        ):
            hs2_shard = dpool.tile([SHARD, F2], DT)
            hs2_full = dpool.tile([NPAD, F2], DT, addr_space="Shared")

            nc.gpsimd.load_library(mlp)

            # ---- constants ----
            w1_sb = cpool.tile([P, 2, 2, P], DT)     # [fin_c, h_c]
            nc.sync.dma_start(out=w1_sb[:], in_=w1_d[:])
            w2_sb = cpool.tile([P, 2, F2], DT)       # [h_c]
            nc.sync.dma_start(out=w2_sb[:], in_=w2_d[:])
            b1_sb = cpool.tile([P, 2], F32)
            nc.sync.dma_start(out=b1_sb[:], in_=b1_d[:])
            b2_sb = cpool.tile([P, 1], F32)
            nc.sync.dma_start(out=b2_sb[:], in_=b2_d[:])
            idx_sb = cpool.tile([P, BPC * 2 * IW], I16)
            nc.sync.dma_start(out=idx_sb[:], in_=idx_d[:])

            # ---- layer 1 + layer-2 table, per dest block ----
            for b in range(BPC):
                xg = xpool.tile([P, NT, FIN], DT, tag="stream")
                nc.sync.dma_start(
                    out=xg[:], in_=xg_d[:, b * NT * FIN:(b + 1) * NT * FIN])
                oh = opool.tile([P, NT, P], DT, tag="oh")
                nc.sync.dma_start(
                    out=oh[:], in_=oh_d[:, b * NT * P:(b + 1) * NT * P])

                aggT0 = pagg.tile([P, P], F32, tag="aggT")
                aggT1 = pagg.tile([P, P], F32, tag="aggT")
                aggTp = [aggT0, aggT1]
                for t in range(NT):
                    for c in range(2):
                        nc.tensor.matmul(
                            aggTp[c][:],
                            lhsT=xg[:, t, c * P:(c + 1) * P],
                            rhs=oh[:, t, :],
                            start=(t == 0), stop=(t == NT - 1))
                aggT_sb = spool.tile([P, 2, P], DT, tag="aggT_sb")
                for c in range(2):
                    nc.vector.tensor_copy(aggT_sb[:, c, :], aggTp[c][:])

                reluT_sb = spool.tile([P, 2, P], DT, tag="reluT")
                for hc in range(2):
                    o1 = po1.tile([P, P], F32, tag="o1")
                    for c in range(2):
                        nc.tensor.matmul(
                            o1[:], lhsT=w1_sb[:, c, hc, :],
                            rhs=aggT_sb[:, c, :],
                            start=(c == 0), stop=(c == 1))
                    nc.scalar.activation(reluT_sb[:, hc, :], o1[:], AF.Relu,
                                         bias=b1_sb[:, hc:hc + 1])

                ph = psm.tile([P, F2], F32, tag="sm")
                for hc in range(2):
                    nc.tensor.matmul(ph[:], lhsT=reluT_sb[:, hc, :],
                                     rhs=w2_sb[:, hc, :],
                                     start=(hc == 0), stop=(hc == 1))
                hsb = spool.tile([P, F2], DT, tag="hsb")
                nc.vector.tensor_copy(hsb[:], ph[:])
                nc.sync.dma_start(out=hs2_shard[b * P:(b + 1) * P, :],
                                  in_=hsb[:])

            # ---- exchange hs2 shards ----
            nc.gpsimd.collective_compute(
                "AllGather", AL.bypass,
                replica_groups=[list(range(NCORES))],
                ins=[hs2_shard[:]],
                outs=[hs2_full[:]],
            )

            # ---- layer 2 per dest block ----
            for b in range(BPC):
                oh2 = opool.tile([P, NT, P], DT, tag="oh")
                nc.sync.dma_start(
                    out=oh2[:], in_=oh_d[:, b * NT * P:(b + 1) * NT * P])
                msgs = []
                for hh in range(2):
                    m = xpool.tile([P, TH, F2], DT, tag="stream")
                    src = hs2_full[0:HALF, :] if hh == 0 else hs2_full[HALF:NPAD, :]
                    k = hh * BPC + b
                    nc.gpsimd.dma_gather(
                        m[:], src,
                        idx_sb[:, k * IW:(k + 1) * IW],
                        CAP, int(cnts[b * 2 + hh]), F2, single_packet=False)
                    msgs.append(m)
                p2 = psm.tile([P, P], F32, tag="sm")
                for t in range(NT):
                    hh, tt = (0, t) if t < TH else (1, t - TH)
                    nc.tensor.matmul(p2[:], lhsT=msgs[hh][:, tt, :],
                                     rhs=oh2[:, t, :],
                                     start=(t == 0), stop=(t == NT - 1))
                o2 = spool.tile([P, P], F32, tag="o2")
                nc.vector.tensor_scalar(o2[:], p2[:], b2_sb[:, 0:1], None,
                                        AL.add)
                nc.sync.dma_start(out=out_d[:, b * P:(b + 1) * P], in_=o2[:])

    nc.compile()
    return nc


def _make_inputs(x, W1, b1, W2, b2, pp):
    TH = pp["TH"]
    CAP = TH * P
    NT = 2 * TH
    IW = CAP // 16

    xp = np.zeros((NPAD, FIN), np.float32)
    xp[:N] = x
    xbf = xp.astype(_BF16)

    # weights in device layouts
    w1c = np.ascontiguousarray(
        W1.astype(_BF16).reshape(2, P, 2, P).transpose(1, 0, 2, 3)
    ).reshape(P, 2 * 2 * P)           # [p, fin_c, h_c, h_lo]
    w2c = np.ascontiguousarray(
        W2.astype(_BF16).reshape(2, P, F2).transpose(1, 0, 2)
    ).reshape(P, 2 * F2)              # [p, h_c, f2]
    b1c = np.ascontiguousarray(
        b1.astype(np.float32).reshape(2, P).T)       # [p, h_c]
    b2c = np.ascontiguousarray(
        b2.astype(np.float32).reshape(P, 1))

    oh = pp["oh"]                     # [NB*2*TH, 128, 128] bf16
    src_slot = pp["src_slot"]         # [NB*2*CAP]
    idx_w = pp["idx_w"]               # [NB, 2, 128, IW]

    in_maps = []
    for c in range(NCORES):
        b0 = c * BPC
        sl = src_slot[b0 * 2 * CAP:(b0 + BPC) * 2 * CAP]
        xg = xbf[np.clip(sl, 0, None)]               # [BPC*NT*128, 256]
        xg = np.ascontiguousarray(
            xg.reshape(BPC * NT, P, FIN).transpose(1, 0, 2)
        ).reshape(P, BPC * NT * FIN)
        ohc = np.ascontiguousarray(
            oh[b0 * NT:(b0 + BPC) * NT].transpose(1, 0, 2)
        ).reshape(P, BPC * NT * P)
        idxc = np.ascontiguousarray(
            idx_w[b0:b0 + BPC].transpose(2, 1, 0, 3)
        ).reshape(P, 2 * BPC * IW)
        in_maps.append({
            "xg": xg, "oh": ohc, "idxP": idxc,
            "w1c": w1c, "w2c": w2c, "b1c": b1c, "b2c": b2c,
        })
    return in_maps


def kernel(x, edge_index, edge_weight, W1, b1, W2, b2, _trace=False):
    from concourse.bass_utils import run_bass_kernel_spmd

    x = np.asarray(x, dtype=np.float32)
    W1 = np.asarray(W1, dtype=np.float32)
    b1 = np.asarray(b1, dtype=np.float32)
    W2 = np.asarray(W2, dtype=np.float32)
    b2 = np.asarray(b2, dtype=np.float32)

    pp = _preprocess(np.asarray(edge_index), np.asarray(edge_weight))
    key = (pp["TH"], tuple(int(v) for v in pp["cnts"]))
    if key not in _NC_CACHE:
        _NC_CACHE[key] = _build_nc(pp["TH"], pp["cnts"])
    nc = _NC_CACHE[key]

    in_maps = _make_inputs(x, W1, b1, W2, b2, pp)
    res = run_bass_kernel_spmd(nc, in_maps, list(range(NCORES)), trace=_trace)
    out = np.concatenate(
        [np.asarray(res.results[c]["out2T"]).T for c in range(NCORES)], axis=0)
    if _trace:
        kernel._last_result = res
    return np.ascontiguousarray(out[:N].astype(np.float32))
